# revision 13
# baseline (speedup 1.0000x reference)
"""TT (tensor-train) AdaptiveRankLinear forward on 8 TRN2 NeuronCores.

Strategy: the TT weight (g1,g2,g3) is tiny (~1 MB) and static, so we
materialize the dense W = TT-contract(g1,g2,g3) on the host (~0.6 GFLOP)
and run the remaining 137 GFLOP x @ W + bias as a data-parallel dense
matmul: batch sharded 8 ways, W (bf16) replicated, each core computing a
512x4096 @ 4096x4096 bf16 matmul with f32 PSUM accumulation.

W and x^T are pre-tiled on the host into the exact [k_tile][n_tile]
[partition][k_subtile][col] blocks the kernel consumes, so every SBUF
tile load is one contiguous 512KB DMA with 4KB-per-partition runs.

Hand-rolled Tile loop (structure from concourse's matmul_tile_kernel):
  for n (8 chunks of 512 out cols):
    for k (8 tiles of 512 contraction rows):
      [n=0: load+cache xT k-tile]  load W (k,n) tile
      for m (4), s (4): matmul into psum[m], accumulating over (k,s)
    for m: psum + bias -> sbuf -> DMA out subtile
Edge optimizations: the (n0,k0) tiles are loaded in 128-row slices so the
first matmul starts after 256KB instead of 1MB; dummy matmuls warm the
PE clock (HAM) during the fill; bias loads lazily after the fill.
"""

import sys

sys.path.insert(0, "/opt/trn_rl_repo")

import numpy as np
import ml_dtypes

B = 4096
D_IN = 4096
D_OUT = 4096
N_CORES = 8
BS = B // N_CORES  # 512 rows per core

P = 128
K_TILE = 512
K_SUB = K_TILE // P  # 4
K_TILES = D_IN // K_TILE  # 8
N_TILE = 512
N_TILES = D_OUT // N_TILE  # 8
M_SUB = BS // P  # 4

N_WARMUP_MM = 7  # dummy matmuls to warm the PE clock during the DMA fill

_CACHE = {}


def _get_nc():
    if "nc" in _CACHE:
        return _CACHE["nc"]

    import concourse.mybir as mybir
    import concourse.tile as tile
    from concourse import bacc
    from concourse.bass import ds

    nc = bacc.Bacc(None, target_bir_lowering=False)
    # pre-tiled layouts: xT[ko][pi][s][m], w[ko][no][pi][s][c]
    xT = nc.declare_dram_parameter(
        "xT", [K_TILES, P, K_SUB, BS], mybir.dt.bfloat16, isOutput=False
    )
    w = nc.declare_dram_parameter(
        "w", [K_TILES, N_TILES, P, K_SUB, N_TILE], mybir.dt.bfloat16, isOutput=False
    )
    biasr = nc.declare_dram_parameter(
        "biasr", [P, D_OUT], mybir.dt.float32, isOutput=False
    )
    out = nc.declare_dram_parameter("out", [BS, D_OUT], mybir.dt.float32, isOutput=True)

    bf16 = mybir.dt.bfloat16
    f32 = mybir.dt.float32

    with tile.TileContext(nc) as tc:
        with (
            tc.tile_pool(name="const", bufs=1) as const_pool,
            tc.tile_pool(name="kxm_pool", bufs=K_TILES - 1) as kxm_pool,
            tc.tile_pool(name="kxm0_pool", bufs=2) as kxm0_pool,
            tc.tile_pool(name="kxn_pool", bufs=3) as kxn_pool,
            tc.tile_pool(name="kxn0_pool", bufs=2) as kxn0_pool,
            tc.tile_pool(name="osb_pool", bufs=4) as osb_pool,
            tc.tile_pool(name="psum", bufs=2, space="PSUM") as psum_pool,
        ):
            # PE warm-up: small dummy matmuls keep the PE HAM activity
            # window busy while the first real tiles are still in flight
            dlhs = const_pool.tile([P, P], bf16)
            drhs = const_pool.tile([P, P], bf16)
            nc.any.memset(dlhs[:], 0.0)
            nc.any.memset(drhs[:], 0.0)
            dps = psum_pool.tile([P, N_TILE], f32, tag="ps0")
            for _ in range(N_WARMUP_MM):
                nc.tensor.matmul(dps[:, :P], dlhs[:], drhs[:], start=True, stop=True)

            bias_sb = const_pool.tile([P, D_OUT], f32)
            out_t = out[:].rearrange("(po pi) f -> pi po f", pi=P)

            # Fill-critical path, all on the sync/HWDGE queue with nothing
            # else competing: the k=0 x tile split 128+384 rows (xa/xb also
            # serve as the permanent k=0 cache), W(0,0) split the same way.
            xa = kxm0_pool.tile([P, 1, BS], bf16, tag="kxm0a")
            wa = kxn0_pool.tile([P, 1, N_TILE], bf16, tag="kxn0a")
            xb = kxm0_pool.tile([P, K_SUB - 1, BS], bf16, tag="kxm0b")
            wb = kxn0_pool.tile([P, K_SUB - 1, N_TILE], bf16, tag="kxn0b")
            nc.sync.dma_start(wa[:], w[0, 0, :, 0:1, :])
            nc.sync.dma_start(xa[:], xT[0, :, 0:1, :])
            nc.sync.dma_start(wb[:], w[0, 0, :, 1:K_SUB, :])
            nc.sync.dma_start(xb[:], xT[0, :, 1:K_SUB, :])

            # Bulk x cache loads, serialized into a chain: a 1-element copy
            # from the previous tile makes each load WAW-wait for it, so at
            # most one bulk x transfer is in flight and the critical k0
            # tiles keep most of the DMA-engine bandwidth during the fill.
            kxm_tiles = {}
            prev = wb
            for k in range(1, K_TILES):
                xt = kxm_pool.tile([P, K_SUB, BS], bf16, tag="kxm", name=f"kxm{k}")
                nc.vector.tensor_copy(out=xt[0:1, 0:1, 0:1], in_=prev[0:1, 0:1, 0:1])
                nc.sync.dma_start(xt[:], xT[k])
                kxm_tiles[k] = xt
                prev = xt

            def lhsT(k, s, m):
                if k == 0:
                    if s == 0:
                        return xa[:, 0, ds(m * P, P)]
                    return xb[:, s - 1, ds(m * P, P)]
                return kxm_tiles[k][:, s, ds(m * P, P)]

            for n in range(N_TILES):
                ps = [
                    psum_pool.tile([P, N_TILE], f32, tag=f"ps{m}", name=f"ps{m}")
                    for m in range(M_SUB)
                ]
                for k in range(K_TILES):
                    if n == 0 and k == 0:
                        for s in range(K_SUB):
                            rhs = wa[:, 0, :] if s == 0 else wb[:, s - 1, :]
                            for m in range(M_SUB):
                                nc.tensor.matmul(
                                    ps[m][:],
                                    lhsT(0, s, m),
                                    rhs,
                                    start=(s == 0),
                                    stop=False,
                                )
                        continue
                    wt = kxn_pool.tile([P, K_SUB, N_TILE], bf16, tag="kxn")
                    nc.sync.dma_start(wt[:], w[k, n])
                    for s in range(K_SUB):
                        for m in range(M_SUB):
                            nc.tensor.matmul(
                                ps[m][:],
                                lhsT(k, s, m),
                                wt[:, s, :],
                                start=(k == 0 and s == 0),
                                stop=(k == K_TILES - 1 and s == K_SUB - 1),
                            )
                if n == 0:
                    nc.gpsimd.dma_start(bias_sb[:], biasr[:])
                # evict: psum + bias -> sbuf, DMA each subtile out
                for m in range(M_SUB):
                    osb = osb_pool.tile([P, N_TILE], f32, tag="osb")
                    nc.vector.tensor_add(
                        out=osb[:],
                        in0=ps[m][:],
                        in1=bias_sb[:, ds(n * N_TILE, N_TILE)],
                    )
                    nc.sync.dma_start(
                        out_t[:, m : m + 1, ds(n * N_TILE, N_TILE)], osb[:, None, :]
                    )
    nc.compile()
    _CACHE["nc"] = nc
    return nc


def _materialize_w(g1, g2, g3):
    # W[(i j k), (n p q)] = sum_{r,s} g1[i,n,r] g2[r,j,p,s] g3[s,k,q]
    W = np.einsum(
        "inr,rjps,skq->ijknpq",
        np.asarray(g1, np.float32),
        np.asarray(g2, np.float32),
        np.asarray(g3, np.float32),
        optimize=True,
    )
    return np.ascontiguousarray(W.reshape(D_IN, D_OUT))


def _make_in_maps(x, g1, g2, g3, bias):
    W = _materialize_w(g1, g2, g3)
    Wb = W.astype(ml_dtypes.bfloat16)
    # [k, n] -> [ko, no, pi, s, c]: row k = ko*K_TILE + s*P + pi
    Wt = np.ascontiguousarray(
        Wb.reshape(K_TILES, K_SUB, P, N_TILES, N_TILE).transpose(0, 3, 2, 1, 4)
    )
    biasr = np.ascontiguousarray(
        np.broadcast_to(np.asarray(bias, np.float32), (P, D_OUT))
    )
    xb = np.asarray(x, np.float32).astype(ml_dtypes.bfloat16)
    in_maps = []
    for c in range(N_CORES):
        xTc = xb[c * BS : (c + 1) * BS, :].T  # [K, BS]
        xTt = np.ascontiguousarray(
            xTc.reshape(K_TILES, K_SUB, P, BS).transpose(0, 2, 1, 3)
        )
        in_maps.append({"xT": xTt, "w": Wt, "biasr": biasr})
    return in_maps


def _run(in_maps, trace=False):
    from concourse.bass_utils import run_bass_kernel_spmd

    nc = _get_nc()
    return run_bass_kernel_spmd(nc, in_maps, core_ids=list(range(N_CORES)), trace=trace)


def kernel(x, g1, g2, g3, bias):
    in_maps = _make_in_maps(x, g1, g2, g3, bias)
    res = _run(in_maps)
    out = np.concatenate(
        [res.results[c]["out"] for c in range(N_CORES)], axis=0
    ).astype(np.float32, copy=False)
    return out


# revision 20
# speedup vs baseline: 1.0565x; 1.0565x over previous
"""Composable-matmul variant (best earlier config: 246.9us run)."""

import sys

sys.path.insert(0, "/opt/trn_rl_repo")

import numpy as np
import ml_dtypes

B = 4096
D_IN = 4096
D_OUT = 4096
N_CORES = 8
BS = B // N_CORES

_CACHE = {}


def _get_nc():
    if "nc" in _CACHE:
        return _CACHE["nc"]

    import concourse.mybir as mybir
    import concourse.tile as tile
    from concourse import bacc
    from concourse.kernels.tile_matmul import (
        composable_matmul_tile_kernel,
        dma_from_dram_kxm,
        dma_from_dram_kxn,
        k_pool_min_bufs,
    )

    nc = bacc.Bacc(None, target_bir_lowering=False)
    xT = nc.declare_dram_parameter("xT", [D_IN, BS], mybir.dt.bfloat16, isOutput=False)
    w = nc.declare_dram_parameter("w", [D_IN, D_OUT], mybir.dt.bfloat16, isOutput=False)
    biasr = nc.declare_dram_parameter(
        "biasr", [128, D_OUT], mybir.dt.float32, isOutput=False
    )
    out = nc.declare_dram_parameter("out", [BS, D_OUT], mybir.dt.float32, isOutput=True)

    with tile.TileContext(nc) as tc:
        num_bufs = k_pool_min_bufs(w[:], max_tile_size=256)
        with (
            tc.tile_pool(name="const", bufs=1) as const_pool,
            tc.tile_pool(name="kxm_pool", bufs=num_bufs) as kxm_pool,
            tc.tile_pool(name="kxn_pool", bufs=num_bufs) as kxn_pool,
        ):
            bias_sb = const_pool.tile([128, D_OUT], mybir.dt.float32)
            out_t = out[:].rearrange("(po pi) f -> pi po f", pi=128)
            bias_loaded = [False]

            def bias_reducer(nc_, psum, sbuf, md):
                if not bias_loaded[0]:
                    nc_.gpsimd.dma_start(bias_sb[:], biasr[:])
                    bias_loaded[0] = True
                sz = md.n_subtile_slice_size
                s = md.n_tile_idx * md.n_tile + md.n_subtile_idx * md.n_subtile
                nc_.vector.tensor_add(
                    out=sbuf[:, :, :sz],
                    in0=psum[:, :sz],
                    in1=bias_sb[: sbuf.shape[0], s : s + sz],
                )
                po = md.m_tile_idx * md.m_subtiles + md.m_subtile_idx
                nc_.sync.dma_start(
                    out_t[:, po : po + 1, s : s + sz], sbuf[:, :, :sz]
                )

            kxm_producer, kxm_shape = dma_from_dram_kxm(kxm_pool, xT[:])
            kxn_producer, kxn_shape = dma_from_dram_kxn(kxn_pool, w[:])

            def mxn_consumer(nc_, mxn_tile, md):
                pass

            composable_matmul_tile_kernel(
                tc=tc,
                kxm_shape=kxm_shape,
                kxn_shape=kxn_shape,
                output_type=mybir.dt.float32,
                kxm_producer=kxm_producer,
                kxn_producer=kxn_producer,
                mxn_consumer=mxn_consumer,
                mxn_subtile_reducer=bias_reducer,
                psum_n_bufs=2,
                MAX_K_TILE_SIZE=256,
            )
    nc.compile()
    _CACHE["nc"] = nc
    return nc


def _materialize_w(g1, g2, g3):
    W = np.einsum(
        "inr,rjps,skq->ijknpq",
        np.asarray(g1, np.float32),
        np.asarray(g2, np.float32),
        np.asarray(g3, np.float32),
        optimize=True,
    )
    return np.ascontiguousarray(W.reshape(D_IN, D_OUT))


def _make_in_maps(x, g1, g2, g3, bias):
    W = _materialize_w(g1, g2, g3)
    Wb = W.astype(ml_dtypes.bfloat16)
    biasr = np.ascontiguousarray(
        np.broadcast_to(np.asarray(bias, np.float32), (128, D_OUT))
    )
    xb = np.asarray(x, np.float32).astype(ml_dtypes.bfloat16)
    in_maps = []
    for c in range(N_CORES):
        xT = np.ascontiguousarray(xb[c * BS : (c + 1) * BS, :].T)
        in_maps.append({"xT": xT, "w": Wb, "biasr": biasr})
    return in_maps


def _run(in_maps, trace=False):
    from concourse.bass_utils import run_bass_kernel_spmd

    nc = _get_nc()
    return run_bass_kernel_spmd(nc, in_maps, core_ids=list(range(N_CORES)), trace=trace)


def kernel(x, g1, g2, g3, bias):
    in_maps = _make_in_maps(x, g1, g2, g3, bias)
    res = _run(in_maps)
    out = np.concatenate(
        [res.results[c]["out"] for c in range(N_CORES)], axis=0
    ).astype(np.float32, copy=False)
    return out
